# revision 11
# baseline (speedup 1.0000x reference)
"""Trainium2 Bass kernel for nn_LogicLayer — fp16 pipeline.

Reference computation:
    p = softmax(weights, axis=-1)            # [OUT, 16]
    c = p @ GATE_COEF                        # [OUT, 4]
    a = x[:, idx0]; b = x[:, idx1]           # [B, OUT]
    out = c0 + c1*a + c2*b + c3*a*b

Strategy (data-parallel over batch, 8 cores, 512 rows each), all fp16
on the wire — halves every DMA stream vs the f32 baseline (80 -> 40
MiB per core; the per-core DMA roofline is ~360 GB/s):
  Host: x -> fp16; fold softmax+coef into ctab; int16 index tables.
  Device, per core:
    Phase 1: stream x shard [512, 8192] fp16, PE-transpose to
             xT [8192, 512] fp16 in DRAM (PSUM->SBUF copies split
             between ACT and DVE).
    Phase 2: dma_gather rows of xT (1 KiB/row); per 128-j slot ACT/DVE
             compute u = c1*a + c0 and v = c3*a + c2 (per-partition
             scale/bias), then half-group fp16 tensor_tensor v*b and
             v+u; store outT fp16. Gather groups are interleaved with
             phase-1 chunks: j's are host-sorted by the highest feature
             chunk their indices touch, so early groups only depend on
             early chunks (row-bounded gather source APs).
  Host: un-permute + transpose per-core outT slices, upcast to f32.
"""

import numpy as np

B, IN_DIM, OUT_DIM = 4096, 8192, 8192
N_CORES = 8
BSH = B // N_CORES  # 512 batch rows per core

GATE_COEF = np.array([
    [0.,  0.,  0.,  0.],
    [0.,  0.,  0.,  1.],
    [0.,  1.,  0., -1.],
    [0.,  1.,  0.,  0.],
    [0.,  0.,  1., -1.],
    [0.,  0.,  1.,  0.],
    [0.,  1.,  1., -2.],
    [0.,  1.,  1., -1.],
    [1., -1., -1.,  1.],
    [1., -1., -1.,  2.],
    [1.,  0., -1.,  0.],
    [1.,  0., -1.,  1.],
    [1., -1.,  0.,  0.],
    [1., -1.,  0.,  1.],
    [1.,  0.,  0., -1.],
    [1.,  0.,  0.,  0.],
], dtype=np.float32)

_NC_CACHE = {}


def sched_levels(nfc=4, ngr=8, jgroup=1024, out_dim=OUT_DIM):
    """Static gather schedule: group g's indices must lie in feature chunks
    0..SCHED[g].  Host sorts j's by max needed chunk; the capacity of level
    l is the count of j's whose indices all fall below chunk l+1, which is
    Binomial(out_dim, ((l+1)/nfc)^2) — take an 8-sigma safety margin."""
    sched = []
    for g in range(ngr):
        need = (g + 1) * jgroup
        lvl = nfc - 1
        for l in range(nfc):
            p = ((l + 1) / nfc) ** 2
            cap = out_dim * p - 8 * np.sqrt(out_dim * p * (1 - p))
            if cap >= need:
                lvl = l
                break
        sched.append(lvl)
    return sched


def build_nc(bsh=BSH, in_dim=IN_DIM, out_dim=OUT_DIM, jgroup=1024, fchunk=2048,
             timing=False, p1_reps=1, p2_reps=1, body_reps=1,
             loop_n=1, loop_body="both"):
    """Build the per-core Bass program (SPMD: same program on all cores).

    timing=True keeps only tiny tensors as external I/O so the per-call
    transfer cost is constant; p1/p2_reps repeat the phases for slope
    timing.
    """
    import concourse.bacc as bacc
    from concourse.bass import AP as BassAP
    import concourse.mybir as mybir
    import concourse.tile as tile
    from concourse.masks import make_identity

    f16 = mybir.dt.float16
    f32 = mybir.dt.float32
    i16 = mybir.dt.int16
    AF = mybir.ActivationFunctionType
    OP = mybir.AluOpType

    nbt = bsh // 128        # batch tiles (partition tiles of x)
    fchunk = min(fchunk, in_dim)
    nfc = in_dim // fchunk  # feature chunks for phase-1 streaming
    nfb_c = fchunk // 128   # feature blocks per chunk
    njb = out_dim // 128    # output-column blocks
    jgroup = min(jgroup, out_dim)
    ngr = out_dim // jgroup  # gather groups
    spg = jgroup // 128      # 128-col slots per group

    nc = bacc.Bacc("TRN2", target_bir_lowering=False, debug=False,
                  num_swdge_queues=2)
    big = "Internal" if timing else None
    x = nc.dram_tensor("x", [bsh, in_dim], f16, kind=big or "ExternalInput")
    ctab = nc.dram_tensor("ctab", [128, njb * 4], f32, kind="ExternalInput")
    idx0w = nc.dram_tensor("idx0w", [128, out_dim // 16], i16, kind="ExternalInput")
    idx1w = nc.dram_tensor("idx1w", [128, out_dim // 16], i16, kind="ExternalInput")
    # each feature chunk stores fchunk+1 rows (one pad row absorbs the
    # pair-token read of "row idx+1" at the chunk boundary)
    xT = nc.dram_tensor("xT", [(in_dim // fchunk) * (fchunk + 1) + 1, bsh],
                        f16, kind="Internal")
    outT = nc.dram_tensor("outT", [out_dim, bsh], f16,
                          kind=big or "ExternalOutput")
    dummy = None
    if timing:
        dummy = nc.dram_tensor("tout", [128, 128], f32, kind="ExternalOutput")

    with tile.TileContext(nc) as tc:
        with (
            tc.tile_pool(name="const", bufs=1) as cpool,
            tc.tile_pool(name="xin", bufs=3) as xpool,
            tc.tile_pool(name="xtout", bufs=2) as xtpool,
            tc.tile_pool(name="psum", bufs=4, space="PSUM") as pspool,
            tc.tile_pool(name="gather", bufs=2) as gpool,
            tc.tile_pool(name="tmp", bufs=2) as tpool,
        ):
            ident = cpool.tile([128, 128], f16)
            make_identity(nc, ident)
            if timing:
                # Internal x is uninitialized DRAM; garbage fp16 (NaN /
                # denormals) makes engines pathologically slow. Fill it
                # with 0.5 once per call (constant cost, cancels in the
                # phase-rep slope).
                xinit = cpool.tile([128, fchunk], f16)
                nc.vector.memset(xinit, 0.5)
                for bt in range(nbt):
                    for fc in range(nfc):
                        nc.sync.dma_start(
                            x[bt * 128:(bt + 1) * 128,
                              fc * fchunk:(fc + 1) * fchunk],
                            xinit,
                        )
            # const loads on the ACT HWDGE queue so the SP queue starts
            # with the first big x chunk DMA immediately
            ctab_sb = cpool.tile([128, njb * 4], f32)
            nc.scalar.dma_start(ctab_sb, ctab[:, :])
            idx0_sb = cpool.tile([128, out_dim // 16], i16)
            nc.scalar.dma_start(idx0_sb, idx0w[:, :])
            idx1_sb = cpool.tile([128, out_dim // 16], i16)
            nc.scalar.dma_start(idx1_sb, idx1w[:, :])

            SCHED = sched_levels(nfc, ngr, jgroup, out_dim)

            def phase1_chunk(fc):
                xin = xpool.tile([128, nbt, fchunk], f16, tag="xin")
                # one big in-DMA per chunk (HWDGE desc-gen is ~650 ns
                # per dma_start on the SP sequencer — batch it)
                nc.sync.dma_start(
                    xin[:, :, :],
                    x[:, fc * fchunk:(fc + 1) * fchunk].rearrange(
                        "(bt p) f -> p bt f", p=128),
                )
                xt_sb = xtpool.tile([128, nfb_c, bsh], f16, tag="xt")
                for fbl in range(nfb_c):
                    fb = fc * nfb_c + fbl
                    ps = pspool.tile([128, nbt * 128], f16, tag="ps")
                    for bt in range(nbt):
                        nc.tensor.transpose(
                            ps[:, bt * 128:(bt + 1) * 128],
                            xin[:, bt, fbl * 128:(fbl + 1) * 128],
                            ident,
                        )
                    # split PSUM->SBUF copies between ACT and DVE
                    if fb % 2 == 0:
                        nc.scalar.copy(xt_sb[:, fbl], ps)
                    else:
                        nc.vector.tensor_copy(xt_sb[:, fbl], ps)
                # p-major row order within the chunk: partition p's
                # nfb_c stripes are contiguous in DRAM -> 16 KiB
                # descriptors (1 KiB rows run ~3x slower on HW)
                c0 = fc * (fchunk + 1)
                og = xT[c0:c0 + fchunk, :].rearrange(
                    "(p s) c -> p s c", s=nfb_c)
                nc.sync.dma_start(og, xt_sb[:, :, :])

            def phase1():
                for fc in range(nfc):
                    phase1_chunk(fc)

            def phase2_group(g):
                icols = jgroup // 16  # idx-table columns per group
                # indices of group g lie in chunks 0..SCHED[g] by host
                # construction — bound the gather source so it only
                # depends on those chunks' xT writes
                # pair tokens: each descriptor covers rows (idx, idx+1)
                # = 2 KiB; 2x gathered bytes but ~3x descriptor rate.
                # Row bound covers only chunks 0..SCHED[g] (incl. pads),
                # so this gather only depends on those chunks' writes.
                rows = (SCHED[g] + 1) * (fchunk + 1)
                xa = xT[:, :]
                src = BassAP(xa.tensor, xa.offset,
                             [[bsh, rows], [1, 2 * bsh]])
                a_sb = gpool.tile([128, spg, bsh], f16, tag="ga")
                b_sb = gpool.tile([128, spg, 2 * bsh], f16, tag="gb")
                # a-tokens are host-sorted by address (near-sequential
                # reads), so plain 1 KiB tokens suffice — no pair waste
                nc.gpsimd.dma_gather(
                    a_sb[:, :, :], xT[0:rows, :],
                    idx0_sb[:, g * icols:(g + 1) * icols],
                    jgroup, jgroup, bsh,
                )
                nc.gpsimd.dma_gather(
                    b_sb[:, :, :], src,
                    idx1_sb[:, g * icols:(g + 1) * icols],
                    jgroup, jgroup, 2 * bsh, elem_step=bsh,
                    queue_num=1,
                )
                o_sb = gpool.tile([128, spg, bsh], f16, tag="go")
                u = tpool.tile([128, spg, bsh], f16, tag="u")
                v = tpool.tile([128, spg, bsh], f16, tag="v")
                hs = spg // 2
                for h in range(2):
                    for s in range(h * hs, (h + 1) * hs):
                        jb = g * spg + s
                        # u = c1*a + c0 on ACT; v = c3*a + c2 alternating
                        # ACT/DVE (per-partition consts force per-slot ops)
                        nc.scalar.activation(
                            u[:, s], a_sb[:, s], AF.Identity,
                            bias=ctab_sb[:, jb * 4 + 0:jb * 4 + 1],
                            scale=ctab_sb[:, jb * 4 + 1:jb * 4 + 2],
                        )
                        if s % 2 == 0:
                            nc.scalar.activation(
                                v[:, s], a_sb[:, s], AF.Identity,
                                bias=ctab_sb[:, jb * 4 + 2:jb * 4 + 3],
                                scale=ctab_sb[:, jb * 4 + 3:jb * 4 + 4],
                            )
                        else:
                            nc.vector.scalar_tensor_tensor(
                                v[:, s], a_sb[:, s],
                                ctab_sb[:, jb * 4 + 3:jb * 4 + 4],
                                ctab_sb[:, jb * 4 + 2:jb * 4 + 3].broadcast_to(
                                    [128, bsh]),
                                OP.mult, OP.add)
                    # half-group fp16 2x TT ops + out-DMA: shorter
                    # dependency chain so the out write starts earlier
                    sl = slice(h * hs, (h + 1) * hs)
                    nc.vector.tensor_tensor(v[:, sl], v[:, sl],
                                            b_sb[:, sl, 0:bsh], OP.mult)
                    nc.vector.tensor_tensor(o_sb[:, sl], v[:, sl], u[:, sl],
                                            OP.add)
                    og = outT[g * jgroup + h * hs * 128:
                              g * jgroup + (h + 1) * hs * 128, :].rearrange(
                        "(p s) c -> p s c", s=hs
                    )
                    nc.sync.dma_start(og, o_sb[:, sl])

            def phase2():
                for g in range(ngr):
                    phase2_group(g)

            def body():
                # interleaved schedule: emit each chunk, then the gather
                # groups whose index range that chunk completes
                for fc in range(nfc):
                    phase1_chunk(fc)
                    for g in range(ngr):
                        if SCHED[g] == fc:
                            phase2_group(g)

            if loop_body == "p1":
                for _ in range(p1_reps):
                    phase1()
            elif loop_body == "p2":
                phase1()
                for _ in range(p2_reps):
                    phase2()
            elif p1_reps > 1 or p2_reps > 1:
                for _ in range(p1_reps):
                    phase1()
                for _ in range(p2_reps):
                    phase2()
            else:
                for _ in range(body_reps):
                    body()

            if dummy is not None:
                nc.sync.dma_start(dummy[:, :], ctab_sb[:, 0:128])

    nc.compile()
    return nc


def host_prep(weights, idx0, idx1, out_dim=OUT_DIM, fchunk=2048, jgroup=1024):
    """Fold softmax+gate coefficients; sort j's by the highest feature
    chunk their indices touch (so early gather groups only depend on
    early transpose chunks); relabel indices to the p-major padded xT
    row layout; build wrapped int16 tables.

    Returns (ctab, idx0w, idx1w, row2j): row2j maps outT rows back to
    output columns (combines the j sort with the p-major out layout)."""
    nfc = IN_DIM // fchunk
    ngr = out_dim // jgroup
    nfb_c = fchunk // 128
    i0 = np.asarray(idx0, dtype=np.int64)
    i1 = np.asarray(idx1, dtype=np.int64)
    cmax = np.maximum(i0, i1) // fchunk
    order = np.argsort(cmax, kind="stable")

    def remap(f):
        # stored xT row: p-major within each (fchunk+1)-row padded chunk
        return ((f // fchunk) * (fchunk + 1) + (f % 128) * nfb_c
                + (f % fchunk) // 128)

    # within each gather group, order j's by the a-token's DRAM address so
    # the gather's token reads are near-sequential (HBM row-buffer locality)
    akey = remap(i0)
    for g in range(out_dim // jgroup):
        sl = slice(g * jgroup, (g + 1) * jgroup)
        order[sl] = order[sl][np.argsort(akey[order[sl]], kind="stable")]
    sched = sched_levels(nfc, ngr, jgroup, out_dim)
    csort = cmax[order]
    for g in range(ngr):
        assert csort[(g + 1) * jgroup - 1] <= sched[g], (
            f"group {g} needs chunk {csort[(g + 1) * jgroup - 1]}, "
            f"schedule allows {sched[g]}"
        )

    w = np.asarray(weights, dtype=np.float32)
    m = w.max(axis=-1, keepdims=True)
    e = np.exp(w - m, dtype=np.float32)
    p = e / e.sum(axis=-1, keepdims=True, dtype=np.float32)
    c = (p @ GATE_COEF).astype(np.float32)[order]  # [out_dim, 4], permuted
    njb = out_dim // 128
    # ctab[p, jb*4+k] = c[jb*128+p, k]
    ctab = np.ascontiguousarray(
        c.reshape(njb, 128, 4).transpose(1, 0, 2).reshape(128, njb * 4)
    )

    def wrap(idx):
        idx = idx.astype(np.int16)
        t = idx.reshape(out_dim // 16, 16).T  # [16, cols]; t[p, col] = idx[col*16+p]
        return np.ascontiguousarray(np.tile(t, (8, 1)))  # replicate to 128 partitions

    # outT row (g, h, p, s_l) -> permuted j (g*jgroup + (h*hs+s_l)*128 + p)
    # -> original column via `order`
    spg = jgroup // 128
    hs = spg // 2
    r = np.arange(out_dim)
    g, rr = r // jgroup, r % jgroup
    h, rr2 = rr // (hs * 128), rr % (hs * 128)
    pp, s_l = rr2 // hs, rr2 % hs
    row2j = order[g * jgroup + (h * hs + s_l) * 128 + pp]

    return ctab, wrap(remap(i0[order])), wrap(remap(i1[order])), row2j


def timing_inputs():
    rng = np.random.default_rng(0)
    w = rng.standard_normal((OUT_DIM, 16)).astype(np.float32)
    i0 = rng.integers(0, IN_DIM, size=OUT_DIM)
    i1 = rng.integers(0, IN_DIM, size=OUT_DIM)
    ctab, i0w, i1w, _ = host_prep(w, i0, i1)
    return {"ctab": ctab, "idx0w": i0w, "idx1w": i1w}


def kernel(x, weights, idx0, idx1):
    from concourse.bass_utils import run_bass_kernel_spmd

    x16 = np.ascontiguousarray(np.asarray(x, dtype=np.float16))
    ctab, i0w, i1w, row2j = host_prep(weights, idx0, idx1)

    if "nc" not in _NC_CACHE:
        _NC_CACHE["nc"] = build_nc()
    nc = _NC_CACHE["nc"]

    in_maps = [
        {
            "x": x16[c * BSH:(c + 1) * BSH],
            "ctab": ctab,
            "idx0w": i0w,
            "idx1w": i1w,
        }
        for c in range(N_CORES)
    ]
    res = run_bass_kernel_spmd(nc, in_maps, core_ids=list(range(N_CORES)))
    out = np.empty((B, OUT_DIM), dtype=np.float32)
    for c in range(N_CORES):
        out[c * BSH:(c + 1) * BSH, row2j] = res.results[c]["outT"].T
    return out
